# revision 53
# baseline (speedup 1.0000x reference)
"""Trainium2 Bass kernel for nn_BinaryGroupConv.

Reference op (per image): BatchNorm2d (inference) -> sign-binarize ->
grouped 3x3 conv (64 groups, 4->4 ch, binarized weights) -> channel
shuffle -> residual add.

Strategy:
  - Data-parallel: 32 images / 8 cores = 4 images per core. No collectives.
  - BN+sign on device: DVE tensor_scalar (x*inv then +t, separately rounded,
    bit-identical to the eager jax reference) then ACT Sign -> bf16 (+-1 and 0
    are exact in bf16).
  - Grouped conv as 9 per-tap block-diagonal matmuls [K=128ch, M=128ch,
    N=464 spatial] accumulated in PSUM. Signed values are exact in bf16, so
    the PE runs at full bf16 rate. Activations are stored in a zero-padded
    58x58 layout so all 9 taps are plain column-shifted slices of one SBUF
    buffer (row-pad columns absorb the horizontal wrap).
  - Channel shuffle is folded into the matmul output-column permutation
    (psum partition m = 32*(c%4) + ((c//4)-32*chunk)), which makes both the
    residual-x gather and the output store simple 32-channel-contiguous DMAs.
  - Residual add fused with the PSUM->SBUF drain on DVE.
"""

import numpy as np

import jax
import ml_dtypes

import concourse.bacc as bacc
import concourse.tile as tile
from concourse import mybir
from contextlib import ExitStack

N_CORES = 8
IMG = 4           # images per core
C = 256
H = W = 56
HP = 58           # padded row length
GRID = HP * HP    # 58x58 padded image
APAD = GRID + 2   # +1 guard element on each side
ROWS_PER_TILE = 8
NT = H // ROWS_PER_TILE          # 7 output tiles per image-chunk
TN = ROWS_PER_TILE * HP          # 464 matmul free dim
TN56 = ROWS_PER_TILE * W         # 448 valid columns per tile
EPS = 1e-5
RHS_MODE = "strided"  # "strided" (N=448, pad cols skipped) or "padded" (N=464)

_CACHE = {}


def _build_program(repeat=1):
    nc = bacc.Bacc("TRN2")
    f32 = mybir.dt.float32
    bf16 = mybir.dt.bfloat16
    x_in = nc.declare_dram_parameter("x", [IMG, C, H, W], f32, isOutput=False)
    wt_in = nc.declare_dram_parameter("wt", [128, 18 * 128], bf16, isOutput=False)
    bn_in = nc.declare_dram_parameter("bn", [128, 4], f32, isOutput=False)
    y_out = nc.declare_dram_parameter("y", [IMG, C, H, W], f32, isOutput=True)

    with tile.TileContext(nc) as tc, ExitStack() as ctx:
        const_pool = ctx.enter_context(tc.tile_pool(name="const", bufs=1))
        apad_pool = ctx.enter_context(tc.tile_pool(name="apad", bufs=1))
        x_pool = ctx.enter_context(tc.tile_pool(name="xin", bufs=3))
        xp_pool = ctx.enter_context(tc.tile_pool(name="xp", bufs=3))
        psum_pool = ctx.enter_context(
            tc.tile_pool(name="ps", bufs=4, space="PSUM")
        )

        # Trigger the ACT table load (Sign/Identity set, ~2.7us) immediately
        # so it overlaps the first DMAs instead of the first real activation.
        warm = const_pool.tile([128, 2], f32, tag="actwarm")
        nc.vector.memset(warm[:], 0.0)
        nc.scalar.activation(warm[:], warm[:], mybir.ActivationFunctionType.Sign)

        bn_sb = const_pool.tile([128, 4], f32, tag="bn")
        nc.sync.dma_start(bn_sb[:], bn_in[:])
        wt_sb = const_pool.tile([128, 18 * 128], bf16, tag="wt")

        apads = []
        for b in range(3):
            ap_t = apad_pool.tile([128, APAD], bf16, tag=f"apad{b}")
            # Zero only the pad cells; ACT rewrites the interior every use.
            nc.vector.memset(ap_t[:, 0:59], 0.0)  # guard + top pad row
            nc.vector.memset(  # right pad of row r | left pad of row r+1 pairs
                ap_t[:, 58 : 58 + 57 * HP].rearrange("p (r z) -> p r z", z=HP)[
                    :, :, 0:2
                ],
                0.0,
            )
            nc.vector.memset(ap_t[:, 1 + 57 * HP : APAD], 0.0)  # bottom + guard
            apads.append(ap_t)

        # Software pipeline, prefetch depth 2: loads(k+1) are emitted a full
        # chunk ahead of bnsign(k) and compute(k-1), so the next chunk's x
        # never queues behind bulk xp/store traffic in the DMA FIFO.
        chunks = [
            (img, c)
            for _rep in range(repeat)
            for img in range(IMG)
            for c in range(2)
        ]
        nc.sync.dma_start(wt_sb[:], wt_in[:])
        signed = [None] * len(chunks)
        for k in range(len(chunks)):
            signed[k] = _emit_prep(nc, k, chunks[k], x_in, bn_sb, apads,
                                   x_pool, xp_pool)
            if k >= 1:
                _emit_compute(nc, signed[k - 1], y_out, wt_sb, psum_pool)
        _emit_compute(nc, signed[-1], y_out, wt_sb, psum_pool, last=True)
    nc.compile()
    return nc


def _emit_prep(nc, k, chunk, x_in, bn_sb, apads, x_pool, xp_pool):
    img, c = chunk
    f32 = mybir.dt.float32
    ap_t = apads[k % 3]
    x_t = x_pool.tile([128, H * W], f32, tag="x")
    # Row-piece pipeline: load / BN-affine / sign per piece, so the first
    # rows' matmuls can start while later rows still prep. Both ACT ops are
    # single-rounded fmas replicating the eager reference's separate mul
    # then add: bit-exact end to end. Chunk 0 preps in quarters so the very
    # first matmuls start ASAP.
    pieces = 4
    rows = H // pieces
    for hh in range(pieces):
        r0 = hh * rows
        sl = slice(r0 * W, (r0 + rows) * W)
        nc.sync.dma_start(
            x_t[:, sl],
            x_in[img, 128 * c : 128 * (c + 1), r0 : r0 + rows, :].rearrange(
                "c h w -> c (h w)"
            ),
        )
        # y = RN(x*inv): fma with zero addend. Keeps DVE free for the adds.
        nc.scalar.activation(
            x_t[:, sl],
            x_t[:, sl],
            mybir.ActivationFunctionType.Identity,
            bias=0.0,
            scale=bn_sb[:, 2 * c : 2 * c + 1],
        )
        interior = ap_t[
            :, 1 + HP * (r0 + 1) + 1 : 1 + HP * (r0 + 1) + 1 + rows * HP
        ].rearrange("p (h w) -> p h w", w=HP)[:, :, 0:W]
        nc.scalar.activation(
            interior,
            x_t[:, sl].rearrange("p (h w) -> p h w", w=W),
            mybir.ActivationFunctionType.Sign,
            bias=bn_sb[:, 2 * c + 1 : 2 * c + 2],
            scale=1.0,
        )
    # Residual x in shuffled channel order (partition 32i+q <- channel
    # 64i+32c+q): 4 large contiguous DMAs. The DVE add accumulates the conv
    # result into this buffer in place; it then doubles as the store source.
    xp = xp_pool.tile([128, H * W], f32, tag="xp")
    for i in range(4):
        ch0 = 64 * i + 32 * c
        nc.sync.dma_start(
            xp[32 * i : 32 * i + 32, :],
            x_in[img, ch0 : ch0 + 32, :, :].rearrange("c h w -> c (h w)"),
        )
    return (img, c, ap_t, xp)


def _emit_compute(nc, stage, y_out, wt_sb, psum_pool, last=False):
    img, c, ap_t, xp = stage
    f32 = mybir.dt.float32
    # Store column groups as soon as their adds are done.
    store_after = {3: (0, 32), 6: (32, 56)}
    ap_grid = ap_t[:, 1 : 1 + HP * HP].rearrange("p (h w) -> p h w", w=HP)
    for t in range(NT):
        if RHS_MODE == "strided":
            ps = psum_pool.tile([128, TN56], f32, tag="ps")
            for tap in range(9):
                dh, dw = tap // 3 - 1, tap % 3 - 1
                r0 = ROWS_PER_TILE * t + 1 + dh
                nc.tensor.matmul(
                    ps[:],
                    wt_sb[:, (9 * c + tap) * 128 : (9 * c + tap + 1) * 128],
                    ap_grid[:, r0 : r0 + ROWS_PER_TILE, 1 + dw : 1 + dw + W],
                    start=(tap == 0),
                    stop=(tap == 8),
                )
            ps_v = ps[:]
        else:
            ps = psum_pool.tile([128, TN], f32, tag="ps")
            for tap in range(9):
                dh, dw = tap // 3 - 1, tap % 3 - 1
                s = 1 + HP * (ROWS_PER_TILE * t + 1 + dh) + dw
                nc.tensor.matmul(
                    ps[:],
                    wt_sb[:, (9 * c + tap) * 128 : (9 * c + tap + 1) * 128],
                    ap_t[:, s : s + TN],
                    start=(tap == 0),
                    stop=(tap == 8),
                )
            ps_v = ps.rearrange("p (h w) -> p h w", w=HP)[:, :, 1 : 1 + W]
        xp_v = xp[:, TN56 * t : TN56 * (t + 1)]
        if RHS_MODE != "strided":
            xp_v = xp_v.rearrange("p (h w) -> p h w", w=W)
        nc.vector.tensor_tensor(xp_v, ps_v, xp_v, op=mybir.AluOpType.add)
        if t in store_after:
            r0, r1 = store_after[t]
            for i in range(4):
                ch0 = 64 * i + 32 * c
                nc.sync.dma_start(
                    y_out[img, ch0 : ch0 + 32, r0:r1, :].rearrange(
                        "c h w -> c (h w)"
                    ),
                    xp[32 * i : 32 * i + 32, r0 * W : r1 * W],
                )


def _pack_weights(weight):
    """Block-diagonal per-tap lhsT tiles with shuffle-folded output order.

    wt[k, (9c+tap)*128 + m]: psum partition m = 32*i + q holds conv output
    channel oc = 128c + 4q + i (group q of chunk c). Nonzero iff input row
    k is in group q (k//4 == q), value sign(weight[oc, k%4, kh, kw]).
    """
    ws = np.sign(weight.astype(np.float32))  # [256, 4, 3, 3]
    wt = np.zeros((128, 2, 9, 128), np.float32)
    q = np.arange(32)
    for c in range(2):
        for tap in range(9):
            kh, kw = tap // 3, tap % 3
            # arr[q, i, j] = ws[128c + 4q + i, j, kh, kw]
            arr = ws[128 * c : 128 * (c + 1), :, kh, kw].reshape(32, 4, 4)
            B = np.zeros((32, 4, 4, 32), np.float32)  # [q, j, i, q']
            B[q, :, :, q] = arr.transpose(0, 2, 1)
            wt[:, c, tap, :] = B.reshape(128, 128)
    return wt.reshape(128, 18 * 128).astype(ml_dtypes.bfloat16)


def _pack_bn(gamma, beta, running_mean, running_var):
    # Mirror the reference ops (and platform) bit-for-bit.
    import jax.numpy as jnp

    inv = np.asarray(
        jnp.asarray(gamma) * jax.lax.rsqrt(jnp.asarray(running_var) + EPS)
    )
    t = np.asarray(jnp.asarray(beta) - jnp.asarray(running_mean) * jnp.asarray(inv))
    bn = np.zeros((128, 4), np.float32)
    bn[:, 0] = inv[0:128]
    bn[:, 1] = t[0:128]
    bn[:, 2] = inv[128:256]
    bn[:, 3] = t[128:256]
    return bn


def _get_runner():
    if "runner" in _CACHE:
        return _CACHE["runner"]
    runner = _make_runner(_build_program())
    _CACHE["runner"] = runner
    return runner


def _make_runner(nc):
    from jax.sharding import Mesh, PartitionSpec, NamedSharding
    from jax.experimental.shard_map import shard_map
    from concourse import bass2jax

    bass2jax.install_neuronx_cc_hook()

    partition_name = (
        nc.partition_id_tensor.name if nc.partition_id_tensor is not None else None
    )
    in_names = []
    out_names = []
    out_avals = []
    for alloc in nc.m.functions[0].allocations:
        if not isinstance(alloc, mybir.MemoryLocationSet):
            continue
        name = alloc.memorylocations[0].name
        if alloc.kind == "ExternalInput":
            if name != partition_name:
                in_names.append(name)
        elif alloc.kind == "ExternalOutput":
            out_names.append(name)
            out_avals.append(
                jax.core.ShapedArray(
                    tuple(alloc.tensor_shape), mybir.dt.np(alloc.dtype)
                )
            )
    n_params = len(in_names)
    bind_in_names = tuple(
        in_names + out_names + ([partition_name] if partition_name else [])
    )

    def _body(*args):
        operands = list(args)
        if partition_name is not None:
            operands.append(bass2jax.partition_id_tensor())
        outs = bass2jax._bass_exec_p.bind(
            *operands,
            out_avals=tuple(out_avals),
            in_names=bind_in_names,
            out_names=tuple(out_names),
            lowering_input_output_aliases=(),
            sim_require_finite=True,
            sim_require_nnan=True,
            nc=nc,
        )
        return tuple(outs)

    devices = jax.devices()[:N_CORES]
    mesh = Mesh(np.asarray(devices), ("core",))
    spec = PartitionSpec("core")
    n_out = len(out_names)
    sharded = jax.jit(
        shard_map(
            _body,
            mesh=mesh,
            in_specs=(spec,) * (n_params + n_out),
            out_specs=(spec,) * n_out,
            check_rep=False,
        ),
        keep_unused=True,
    )
    sharding = NamedSharding(mesh, spec)
    zeros = [
        jax.device_put(
            np.zeros((N_CORES * a.shape[0], *a.shape[1:]), a.dtype), sharding
        )
        for a in out_avals
    ]
    return dict(
        nc=nc,
        fn=sharded,
        in_names=in_names,
        out_names=out_names,
        sharding=sharding,
        zeros=zeros,
    )


def _device_inputs(x, weight, gamma, beta, running_mean, running_var):
    """Host-side packing -> concatenated per-core arrays on the 8 devices."""
    r = _get_runner()
    wt = np.asarray(_pack_weights(np.asarray(weight, np.float32)))
    bn = _pack_bn(
        np.asarray(gamma, np.float32),
        np.asarray(beta, np.float32),
        np.asarray(running_mean, np.float32),
        np.asarray(running_var, np.float32),
    )
    x = np.ascontiguousarray(np.asarray(x, np.float32))
    concat = {
        "x": x.reshape(N_CORES * IMG, C, H, W),
        "wt": np.concatenate([wt] * N_CORES, axis=0),
        "bn": np.concatenate([bn] * N_CORES, axis=0),
    }
    args = [
        jax.device_put(concat[name], r["sharding"]) for name in r["in_names"]
    ]
    return r, args


def kernel(x, weight, gamma, beta, running_mean, running_var):
    r, args = _device_inputs(x, weight, gamma, beta, running_mean, running_var)
    outs = r["fn"](*args, *r["zeros"])
    y = np.asarray(outs[0])
    return y.reshape(N_CORES * IMG, C, H, W)


def bench(x, weight, gamma, beta, running_mean, running_var, iters=30):
    """Steady-state per-call wall time (s) with device-resident inputs."""
    import time

    r, args = _device_inputs(x, weight, gamma, beta, running_mean, running_var)
    out = r["fn"](*args, *r["zeros"])
    jax.block_until_ready(out)
    t0 = time.perf_counter()
    for _ in range(iters):
        out = r["fn"](*args, *r["zeros"])
    jax.block_until_ready(out)
    dt = (time.perf_counter() - t0) / iters
    return dt, np.asarray(out[0]).reshape(N_CORES * IMG, C, H, W)


def _time_runner(r, args, iters):
    import time

    out = r["fn"](*args, *r["zeros"])
    jax.block_until_ready(out)
    best = float("inf")
    for _ in range(3):
        t0 = time.perf_counter()
        for _ in range(iters):
            out = r["fn"](*args, *r["zeros"])
        jax.block_until_ready(out)
        best = min(best, (time.perf_counter() - t0) / iters)
    return best, out


def measure_hw_time(
    x, weight, gamma, beta, running_mean, running_var, r_hi=5, iters=40
):
    """Per-launch HW time via repeat-factor slope: T = (t(R) - t(1)) / (R-1).

    Immune to the axon dispatch floor. Returns (hw_seconds, output).
    """
    r1, args = _device_inputs(x, weight, gamma, beta, running_mean, running_var)
    key = f"runner_rep{r_hi}"
    if key not in _CACHE:
        _CACHE[key] = _make_runner(_build_program(repeat=r_hi))
    rH = _CACHE[key]
    t1, out1 = _time_runner(r1, args, iters)
    tH, outH = _time_runner(rH, args, iters)
    hw = (tH - t1) / (r_hi - 1)
    y = np.asarray(out1[0]).reshape(N_CORES * IMG, C, H, W)
    yH = np.asarray(outH[0]).reshape(N_CORES * IMG, C, H, W)
    assert np.array_equal(y, yH), "repeat variant output mismatch"
    return hw, t1, tH, y
